# revision 11
# baseline (speedup 1.0000x reference)
"""Bass/Tile Trainium2 kernel for nn_ConstrainedAttention (B=2,S=2048,D=512,H=8).

Sharding: 8 cores = 2 batches x 4 head-pairs. Core c handles batch b=c//4 and
heads (2*(c%4), 2*(c%4)+1). Each core computes its heads' attention plus the
output-projection partial; the host sums the 4 partials per batch and adds bo.

Device-side layout is "scores transposed": [k on partitions, q on free dim].
Softmax is computed without a max-subtraction pass (scores are bounded, fp32
exp cannot overflow) and the softmax denominator falls out of the attn@V
matmul via an appended ones-column on V. Position bias below -POS_CUT makes
exp() vanish at fp32 precision, so score tiles entirely outside the band are
skipped (the sparse-attention structure of this problem).
"""

import sys

sys.path.insert(0, "/opt/trn_rl_repo")

import numpy as np

import bass_rust
import concourse.bass as bass
import concourse.tile as tile
from concourse import mybir
from concourse.masks import make_identity
from concourse.vector_clock import ScopedClock

# ---- problem constants (hardcoded per contract) ----
B, S, D, H, HD, DSEM = 2, 2048, 512, 8, 64, 256
P = 128
NCORES = 8
SEM_THRESH = 0.5
SEM_STRENGTH = 1.0
POS_WINDOW = 10.0
POS_DECAY = 0.1
TEMP_MIN, TEMP_MAX = 0.5, 2.0

QC = 4          # q chunks of 512
KC = S // P     # 16 k chunks of 128
QW = 512        # q chunk width
# Skip (kc,qc) score tiles whose minimum |q-k| distance puts pos_bias below
# -POS_CUT: exp(score + pos) is then < e-50 relative to the softmax sum.
POS_CUT = 60.0

F32 = mybir.dt.float32
F32R = mybir.dt.float32r
AX = mybir.AxisListType.X
ALU = mybir.AluOpType
ACTF = mybir.ActivationFunctionType


def _tile_kept(kc: int, qc: int) -> bool:
    """Does score tile (k in [kc*128,kc*128+128), q in [qc*512,qc*512+512))
    intersect the band where pos_bias > -POS_CUT?"""
    dmax = POS_WINDOW + POS_CUT / POS_DECAY  # distance where bias hits -POS_CUT
    k0, k1 = kc * P, kc * P + P - 1
    q0, q1 = qc * QW, qc * QW + QW - 1
    # min |q - k| over the tile
    if q0 <= k1 and k0 <= q1:
        dmin = 0
    else:
        dmin = min(abs(q0 - k1), abs(k0 - q1))
    return dmin <= dmax


def _kept_deltas() -> list[int]:
    ds = sorted({qc * QW - kc * P for qc in range(QC) for kc in range(KC)
                 if _tile_kept(kc, qc)})
    return ds


def _pos_tile(delta: int) -> np.ndarray:
    """pos_biasT tile [128 k, 512 q] for q0-k0 == delta:
    t[dk, dq] = g(delta + dq - dk), g(d) = min(0, -0.1*(|d|-10))."""
    dk = np.arange(P)[:, None]
    dq = np.arange(QW)[None, :]
    d = np.abs(delta + dq - dk).astype(np.float32)
    return np.where(d > POS_WINDOW, -POS_DECAY * (d - POS_WINDOW), 0.0).astype(
        np.float32
    )


def _bcast(ap, p):
    """Broadcast an AP along the partition dim (step 0, count p)."""
    return bass.AP(tensor=ap.tensor, offset=ap.offset, ap=[[0, p]] + ap.ap[1:])


def _patched_drain_and_barrier(self, tick_clock, wait_clock):
    """The walrus build in this container rejects >1 sem wait on TPB_CTRL
    instructions (Drain/Nop). Spread the tile-exit waits one-per-nop."""
    nop_inst = self.nc.sync.nop(nofuse=True, hint="tile_exit_wait")
    wait_clock.add_sem_waits(
        nop_inst.ins, ScopedClock({None: tick_clock.global_clock})
    )
    waits = list(nop_inst.ins.sync_info.on_wait)
    nop_inst.ins.sync_info.on_wait = waits[:1]
    for w in waits[1:]:
        extra = self.nc.sync.nop(nofuse=True, hint="tile_exit_wait")
        extra.ins.sync_info = bass_rust.SyncInfo(on_wait=[w], on_update=[])
    self.nc.sync.drain()
    self.nc.all_engine_barrier()
    popped = self.nc._tile_sem_poison_stack.pop()
    assert popped is self._sem_poison
    self.nc.clear_and_free_semaphores(list(self.sems.allocated().values()))
    self.nc.all_engine_barrier()


tile.TileContext._drain_and_barrier = _patched_drain_and_barrier


def _split_multi_waits_json(raw: bytes) -> bytes:
    """This container's walrus accepts at most ONE semaphore wait per
    instruction (setupSyncWait: 'Too many sync wait commands'). Rewrite the
    serialized BIR: for every instruction carrying N>1 waits, hoist N-1 of
    them onto same-engine NoOps inserted immediately before it."""
    import json as _json

    d = _json.loads(raw)
    seq = [0]
    for fn in d["functions"]:
        for bb in fn["blocks"]:
            new_insts = []
            for ins in bb["instructions"]:
                si = ins.get("sync_info")
                waits = (si or {}).get("on_wait") or []
                if len(waits) > 1:
                    for w in waits[:-1]:
                        seq[0] += 1
                        new_insts.append({
                            "debug": ins.get("debug", 0),
                            "engine": ins["engine"],
                            "ins": [],
                            "outs": [],
                            "name": f"I-w{seq[0]}",
                            "opcode": "NoOp",
                            "sync_info": {"on_update": [], "on_wait": [w]},
                            "text_hint": "split_wait",
                        })
                    si["on_wait"] = [waits[-1]]
                new_insts.append(ins)
            bb["instructions"] = new_insts
    return _json.dumps(d).encode()


_orig_to_json_bytes = bass.Bass.to_json_bytes


def _to_json_bytes_split(self, *a, **kw):
    return _split_multi_waits_json(_orig_to_json_bytes(self, *a, **kw))


bass.Bass.to_json_bytes = _to_json_bytes_split


def r32(x):
    return x.bitcast(F32R)


def build_nc() -> bass.Bass:
    """Build the per-core Bass program (identical on all 8 cores)."""
    nc = bass.Bass()
    deltas = _kept_deltas()
    didx = {d: i for i, d in enumerate(deltas)}
    ND = len(deltas)

    # ---- DRAM I/O ----
    qT = nc.dram_tensor("qT", [D, S], F32, kind="ExternalInput")
    kT = nc.dram_tensor("kT", [D, S], F32, kind="ExternalInput")
    vT = nc.dram_tensor("vT", [D, S], F32, kind="ExternalInput")
    semT = nc.dram_tensor("semT", [DSEM, S], F32, kind="ExternalInput")
    maskc = nc.dram_tensor("maskc", [P, KC], F32, kind="ExternalInput")
    wq = nc.dram_tensor("wq", [D, P], F32, kind="ExternalInput")
    wk = nc.dram_tensor("wk", [D, P], F32, kind="ExternalInput")
    wv = nc.dram_tensor("wv", [D, P], F32, kind="ExternalInput")
    wo = nc.dram_tensor("wo", [P, D], F32, kind="ExternalInput")
    bqc = nc.dram_tensor("bqc", [P, 1], F32, kind="ExternalInput")
    bkc = nc.dram_tensor("bkc", [P, 1], F32, kind="ExternalInput")
    bvc = nc.dram_tensor("bvc", [P, 1], F32, kind="ExternalInput")
    wt1T = nc.dram_tensor("wt1T", [D, D // 2], F32, kind="ExternalInput")
    bt1r = nc.dram_tensor("bt1r", [1, D // 2], F32, kind="ExternalInput")
    wt2r = nc.dram_tensor("wt2r", [1, D // 2], F32, kind="ExternalInput")
    bt2s = nc.dram_tensor("bt2s", [1, 1], F32, kind="ExternalInput")
    posb = nc.dram_tensor("posb", [ND, P, QW], F32, kind="ExternalInput")
    out = nc.dram_tensor("out", [S, D], F32, kind="ExternalOutput")

    with nc.allow_low_precision(reason="f32r-rounded matmul inputs"), \
            tile.TileContext(nc) as tc:
        with (
            tc.tile_pool(name="persist", bufs=1) as pp,
            tc.tile_pool(name="stream", bufs=4) as st,
            tc.tile_pool(name="work", bufs=3) as wk_pool,
            tc.tile_pool(name="psum_big", bufs=3, space="PSUM") as pb,
            tc.tile_pool(name="psum_sim", bufs=2, space="PSUM") as ps,
            tc.tile_pool(name="psum_small", bufs=1, space="PSUM") as p1,
            tc.tile_pool(name="psum_ctx", bufs=2, space="PSUM") as pc,
            tc.tile_pool(name="dram", bufs=1, space="DRAM") as dram,
        ):
            # ---- persistent SBUF ----
            sn_sb = pp.tile([P, 2, S], F32)            # semantic.T (2 chunks)
            pos_sb = pp.tile([P, ND, QW], F32)         # pos-bias tiles by delta
            q0_sb = pp.tile([HD, S], F32)              # head0 Q.T (scaled)
            q1_sb = pp.tile([HD, S], F32)
            k0_sb = pp.tile([HD, S], F32)
            k1_sb = pp.tile([HD, S], F32)
            v0_sb = pp.tile([P, KC, HD + 1], F32)      # V per k-chunk + ones col
            v1_sb = pp.tile([P, KC, HD + 1], F32)
            ctx_sb = pp.tile([P, S], F32)              # both heads' ctx.T
            mask_sb = pp.tile([P, KC], F32)
            wq_sb = pp.tile([P, 4, P], F32)
            wk_sb = pp.tile([P, 4, P], F32)
            wv_sb = pp.tile([P, 4, P], F32)
            wo_sb = pp.tile([P, D], F32)
            bq_sb = pp.tile([P, 1], F32)
            bk_sb = pp.tile([P, 1], F32)
            bv_sb = pp.tile([P, 1], F32)
            wt1_sb = pp.tile([P, 4, D // 2], F32)
            bt1_sb = pp.tile([1, D // 2], F32)
            wt2_sb = pp.tile([1, D // 2], F32)
            bt2_sb = pp.tile([1, 1], F32)
            ident = pp.tile([P, P], F32)
            ones_sb = pp.tile([P, 1], F32)
            ones_row = pp.tile([1, P], F32)
            scale_col = pp.tile([P, 1], F32)
            qsum_sb = pp.tile([P, 4], F32)
            rnorm_sb = pp.tile([1, S], F32)
            scale_sb = pp.tile([1, 1], F32)            # 1/(8*temp)
            bqs_sb = pp.tile([P, 1], F32)              # bq * scale
            mlp_sb = pp.tile([1, D // 2], F32)
            tsig_sb = pp.tile([1, 1], F32)

            # ---- constant/small loads ----
            nc.sync.dma_start(out=mask_sb, in_=maskc[:])
            nc.sync.dma_start(out=r32(wq_sb[:]), in_=r32(wq.rearrange("(c p) m -> p c m", p=P)))
            nc.sync.dma_start(out=r32(wk_sb[:]), in_=r32(wk.rearrange("(c p) m -> p c m", p=P)))
            nc.sync.dma_start(out=r32(wv_sb[:]), in_=r32(wv.rearrange("(c p) m -> p c m", p=P)))
            nc.sync.dma_start(out=r32(wo_sb[:]), in_=r32(wo[:]))
            nc.sync.dma_start(out=bq_sb, in_=bqc[:])
            nc.sync.dma_start(out=bk_sb, in_=bkc[:])
            nc.sync.dma_start(out=bv_sb, in_=bvc[:])
            nc.sync.dma_start(
                out=r32(wt1_sb[:]), in_=r32(wt1T.rearrange("(c p) m -> p c m", p=P))
            )
            nc.sync.dma_start(out=bt1_sb, in_=bt1r[:])
            nc.sync.dma_start(out=wt2_sb, in_=wt2r[:])
            nc.sync.dma_start(out=bt2_sb, in_=bt2s[:])
            nc.sync.dma_start(out=pos_sb, in_=posb.rearrange("n p m -> p n m"))
            nc.sync.dma_start(
                out=r32(sn_sb[:]), in_=r32(semT.rearrange("(c p) m -> p c m", p=P))
            )
            make_identity(nc, ident)
            nc.vector.memset(ones_sb, 1.0)
            nc.vector.memset(ones_row, 1.0)
            nc.vector.memset(v0_sb[:, :, HD : HD + 1], 1.0)
            nc.vector.memset(v1_sb[:, :, HD : HD + 1], 1.0)

            # ---- semantic feature normalization ----
            # norms^2 per column via square + ones-matmul, then rsqrt, then
            # scale sn in place.
            for qs in range(QC):
                n2 = p1.tile([1, QW], F32, tag="small")
                for c in range(2):
                    sq = wk_pool.tile([P, QW], F32, tag="sq")
                    nc.vector.tensor_mul(
                        r32(sq[:]),
                        sn_sb[:, c, qs * QW : (qs + 1) * QW],
                        sn_sb[:, c, qs * QW : (qs + 1) * QW],
                    )
                    nc.tensor.matmul(
                        n2, r32(ones_sb), r32(sq), start=(c == 0), stop=(c == 1)
                    )
                nrm = wk_pool.tile([1, QW], F32, tag="nrm")
                nc.scalar.activation(nrm, n2, ACTF.Sqrt)
                nc.vector.reciprocal(r32(rnorm_sb[:, qs * QW : (qs + 1) * QW]), nrm)
            for qs in range(QC):
                qsl = slice(qs * QW, (qs + 1) * QW)
                rb = ps.tile([P, QW], F32, tag="sim", name=f"rb{qs}")
                nc.tensor.matmul(
                    rb, r32(ones_row), r32(rnorm_sb[:, qsl]),
                    start=True, stop=True,
                )
                for c in range(2):
                    nc.vector.tensor_mul(r32(sn_sb[:, c, qsl]), sn_sb[:, c, qsl], rb)

            # ---- Q/K/V projections (stream x.T chunks over D) ----
            # Q also feeds the temperature MLP via per-chunk row sums.
            xqs = [st.tile([P, S], F32, tag="xT", name=f"xq{i}") for i in range(4)]
            for kc in range(4):
                nc.sync.dma_start(out=r32(xqs[kc][:]), in_=r32(qT[kc * P : (kc + 1) * P, :]))
                nc.vector.reduce_sum(r32(qsum_sb[:, kc : kc + 1]), xqs[kc], axis=AX)

            # temperature MLP: sigmoid(relu(qm@Wt1.T+bt1)@Wt2.T+bt2)
            h1p = p1.tile([1, D // 2], F32, tag="small")
            for kc in range(4):
                nc.tensor.matmul(
                    h1p,
                    r32(qsum_sb[:, kc : kc + 1]),
                    r32(wt1_sb[:, kc, :]),
                    start=(kc == 0),
                    stop=(kc == 3),
                )
            nc.vector.tensor_scalar(
                mlp_sb, h1p, 1.0 / S, None, op0=ALU.mult
            )
            nc.vector.tensor_add(mlp_sb, mlp_sb, bt1_sb)
            nc.scalar.activation(mlp_sb, mlp_sb, ACTF.Relu)
            nc.vector.tensor_mul(mlp_sb, mlp_sb, wt2_sb)
            nc.vector.reduce_sum(tsig_sb, mlp_sb, axis=AX)
            nc.scalar.activation(tsig_sb, tsig_sb, ACTF.Sigmoid, bias=bt2_sb)
            # scale = 1/(sqrt(HD)*temp) = 1/(8*(0.5+1.5*sig)) = 1/(12*sig+4)
            nc.vector.tensor_scalar(
                tsig_sb, tsig_sb, 12.0, 4.0, op0=ALU.mult, op1=ALU.add
            )
            nc.vector.reciprocal(r32(scale_sb[:]), tsig_sb)
            # broadcast the scalar to all partitions via a DRAM bounce (SBUF
            # sources cannot have partition-step-0 APs; DRAM sources can)
            scale_dr = dram.tile([1, 1], F32)
            nc.sync.dma_start(out=scale_dr, in_=scale_sb)
            nc.sync.dma_start(out=scale_col, in_=_bcast(scale_dr[:], P))
            nc.vector.tensor_scalar(
                bqs_sb, bq_sb, scale_col, None, op0=ALU.mult
            )

            # Q = x@Wq per q-chunk; evict with (x + bq) * scale fused:
            # out = in*scale + bq*scale
            for qs in range(QC):
                qp = pb.tile([P, QW], F32, tag="big", name=f"qp{qs}")
                for kc in range(4):
                    nc.tensor.matmul(
                        qp,
                        r32(wq_sb[:, kc, :]),
                        r32(xqs[kc][:, qs * QW : (qs + 1) * QW]),
                        start=(kc == 0),
                        stop=(kc == 3),
                    )
                nc.scalar.activation(
                    r32(q0_sb[:, qs * QW : (qs + 1) * QW]),
                    qp[0:HD, :],
                    ACTF.Identity,
                    bias=bqs_sb[0:HD, :],
                    scale=scale_col[0:HD, :],
                )
                nc.scalar.activation(
                    r32(q1_sb[:, qs * QW : (qs + 1) * QW]),
                    qp[HD:P, :],
                    ACTF.Identity,
                    bias=bqs_sb[HD:P, :],
                    scale=scale_col[HD:P, :],
                )

            xks = [st.tile([P, S], F32, tag="xT", name=f"xk{i}") for i in range(4)]
            for kc in range(4):
                nc.sync.dma_start(out=r32(xks[kc][:]), in_=r32(kT[kc * P : (kc + 1) * P, :]))
            for qs in range(QC):
                kp = pb.tile([P, QW], F32, tag="big", name=f"kp{qs}")
                for kc in range(4):
                    nc.tensor.matmul(
                        kp,
                        r32(wk_sb[:, kc, :]),
                        r32(xks[kc][:, qs * QW : (qs + 1) * QW]),
                        start=(kc == 0),
                        stop=(kc == 3),
                    )
                nc.scalar.activation(
                    r32(k0_sb[:, qs * QW : (qs + 1) * QW]),
                    kp[0:HD, :],
                    ACTF.Identity,
                    bias=bk_sb[0:HD, :],
                )
                nc.scalar.activation(
                    r32(k1_sb[:, qs * QW : (qs + 1) * QW]),
                    kp[HD:P, :],
                    ACTF.Identity,
                    bias=bk_sb[HD:P, :],
                )

            # V: project to vT layout then transpose per 128-chunk into
            # [k-part, head-dim] with the ones column for the softmax sum.
            vtmp_sb = pp.tile([P, S], F32)
            xvs = [st.tile([P, S], F32, tag="xT", name=f"xv{i}") for i in range(4)]
            for kc in range(4):
                nc.sync.dma_start(out=r32(xvs[kc][:]), in_=r32(vT[kc * P : (kc + 1) * P, :]))
            for qs in range(QC):
                vp = pb.tile([P, QW], F32, tag="big", name=f"vp{qs}")
                for kc in range(4):
                    nc.tensor.matmul(
                        vp,
                        r32(wv_sb[:, kc, :]),
                        r32(xvs[kc][:, qs * QW : (qs + 1) * QW]),
                        start=(kc == 0),
                        stop=(kc == 3),
                    )
                nc.scalar.activation(
                    vtmp_sb[:, qs * QW : (qs + 1) * QW],
                    vp,
                    ACTF.Identity,
                    bias=bv_sb,
                )
            for sc in range(KC):
                vtp = p1.tile([P, P], F32, tag="small", name=f"vtp{sc}")
                nc.tensor.transpose(
                    vtp, vtmp_sb[:, sc * P : (sc + 1) * P], ident
                )
                nc.scalar.copy(r32(v0_sb[:, sc, 0:HD]), vtp[:, 0:HD])
                nc.scalar.copy(r32(v1_sb[:, sc, 0:HD]), vtp[:, HD:P])

            # ---- main attention loop ----
            for qc in range(QC):
                cx0 = pc.tile([HD + 1, QW], F32, tag="ctx")
                cx1 = pc.tile([HD + 1, QW], F32, tag="ctx")
                kept = [kc for kc in range(KC) if _tile_kept(kc, qc)]
                for kc in kept:
                    first = kc == kept[0]
                    last = kc == kept[-1]
                    d = qc * QW - kc * P
                    qsl = slice(qc * QW, (qc + 1) * QW)
                    ksl = slice(kc * P, (kc + 1) * P)
                    # semantic bias tile: min(sim-0.5, 0) + pos
                    smp = ps.tile([P, QW], F32, tag="sim")
                    for c in range(2):
                        nc.tensor.matmul(
                            smp,
                            r32(sn_sb[:, c, ksl]),
                            r32(sn_sb[:, c, qsl]),
                            start=(c == 0),
                            stop=(c == 1),
                        )
                    bias = wk_pool.tile([P, QW], F32, tag="bias")
                    nc.vector.tensor_scalar(
                        bias, smp, SEM_THRESH, SEM_THRESH,
                        op0=ALU.min, op1=ALU.subtract,
                    )
                    nc.vector.tensor_add(bias, bias, pos_sb[:, didx[d], :])
                    for h, (qh, kh, vh, cx) in enumerate(
                        ((q0_sb, k0_sb, v0_sb, cx0), (q1_sb, k1_sb, v1_sb, cx1))
                    ):
                        scp = pb.tile([P, QW], F32, tag="big")
                        nc.tensor.matmul(
                            scp, r32(kh[:, ksl]), r32(qh[:, qsl]),
                            start=True, stop=True,
                        )
                        tt = wk_pool.tile([P, QW], F32, tag="tt")
                        nc.vector.tensor_add(tt, scp, bias)
                        ee = wk_pool.tile([P, QW], F32, tag="ee")
                        nc.scalar.activation(
                            r32(ee[:]), tt, ACTF.Exp, bias=mask_sb[:, kc : kc + 1]
                        )
                        nc.tensor.matmul(
                            cx, r32(vh[:, kc, :]), r32(ee),
                            start=first, stop=last,
                        )
                # normalize: ctx /= sum (sum = ones-row of the V matmul)
                for h, cx in enumerate((cx0, cx1)):
                    rec = wk_pool.tile([1, QW], F32, tag="rec")
                    nc.vector.reciprocal(r32(rec[:]), cx[HD : HD + 1, :])
                    rcb = ps.tile([HD, QW], F32, tag="sim", name=f"rcb{qc}_{h}")
                    nc.tensor.matmul(
                        rcb, r32(ones_row[:, 0:HD]), r32(rec),
                        start=True, stop=True,
                    )
                    rcs = wk_pool.tile([HD, QW], F32, tag="rcs")
                    nc.scalar.copy(rcs, rcb)
                    nc.vector.tensor_mul(
                        r32(ctx_sb[h * HD : (h + 1) * HD, qc * QW : (qc + 1) * QW]),
                        cx[0:HD, :],
                        rcs,
                    )

            # ---- output projection partial: out = ctx.T @ wo ----
            for sc in range(KC):
                op = pb.tile([P, D], F32, tag="big", name=f"op{sc}")
                nc.tensor.matmul(
                    op,
                    r32(ctx_sb[:, sc * P : (sc + 1) * P]),
                    r32(wo_sb),
                    start=True,
                    stop=True,
                )
                ob = wk_pool.tile([P, D], F32, tag="ob")
                nc.scalar.copy(ob, op)
                nc.sync.dma_start(out=out[sc * P : (sc + 1) * P, :], in_=ob)

    return nc


# ---------------------------------------------------------------- host side

_CACHE: dict = {}


def _shard_inputs(inputs: dict) -> list[dict]:
    f = np.float32
    q = np.asarray(inputs["query"], f)
    k = np.asarray(inputs["key"], f)
    v = np.asarray(inputs["value"], f)
    mask = np.asarray(inputs["mask"])
    sem = np.asarray(inputs["semantic_features"], f)
    Wq, bq = np.asarray(inputs["Wq"], f), np.asarray(inputs["bq"], f)
    Wk, bk = np.asarray(inputs["Wk"], f), np.asarray(inputs["bk"], f)
    Wv, bv = np.asarray(inputs["Wv"], f), np.asarray(inputs["bv"], f)
    Wo = np.asarray(inputs["Wo"], f)
    Wt1, bt1 = np.asarray(inputs["Wt1"], f), np.asarray(inputs["bt1"], f)
    Wt2, bt2 = np.asarray(inputs["Wt2"], f), np.asarray(inputs["bt2"], f)

    deltas = _kept_deltas()
    posb = np.stack([_pos_tile(d) for d in deltas])  # [ND,128,512]
    wt1T = np.ascontiguousarray(Wt1.T)               # [512,256]

    maps = []
    for c in range(NCORES):
        b, hp = divmod(c, 4)
        cols = slice(P * hp, P * hp + P)
        maskbias = np.where(mask[b] == 0, np.float32(-1e30), np.float32(0.0))
        maps.append({
            "qT": np.ascontiguousarray(q[b].T),
            "kT": np.ascontiguousarray(k[b].T),
            "vT": np.ascontiguousarray(v[b].T),
            "semT": np.ascontiguousarray(sem[b].T),
            "maskc": np.ascontiguousarray(maskbias.reshape(KC, P).T),
            "wq": np.ascontiguousarray(Wq[cols, :].T),
            "wk": np.ascontiguousarray(Wk[cols, :].T),
            "wv": np.ascontiguousarray(Wv[cols, :].T),
            "wo": np.ascontiguousarray(Wo[:, cols].T),
            "bqc": np.ascontiguousarray(bq[cols].reshape(P, 1)),
            "bkc": np.ascontiguousarray(bk[cols].reshape(P, 1)),
            "bvc": np.ascontiguousarray(bv[cols].reshape(P, 1)),
            "wt1T": wt1T,
            "bt1r": bt1.reshape(1, -1),
            "wt2r": Wt2.reshape(1, -1),
            "bt2s": bt2.reshape(1, 1),
            "posb": posb,
        })
    return maps


def _gather(results: list[dict], inputs: dict) -> np.ndarray:
    bo = np.asarray(inputs["bo"], np.float32)
    out = np.zeros((B, S, D), np.float32)
    for c in range(NCORES):
        out[c // 4] += results[c]["out"]
    out += bo
    return out


def get_nc() -> bass.Bass:
    if "nc" not in _CACHE:
        _CACHE["nc"] = build_nc()
    return _CACHE["nc"]


def _get_runner():
    """Compile once and return a reusable 8-core runner (the stock
    run_bass_via_pjrt re-jits per call, recompiling the NEFF each time)."""
    if "runner" in _CACHE:
        return _CACHE["runner"]

    import jax
    from jax.sharding import Mesh, PartitionSpec
    from jax.experimental.shard_map import shard_map
    from concourse import bass2jax, mybir as mb

    nc = get_nc()
    bass2jax.install_neuronx_cc_hook()

    in_names, out_names, out_avals, zero_shapes = [], [], [], []
    partition_name = (
        nc.partition_id_tensor.name if nc.partition_id_tensor else None
    )
    for alloc in nc.m.functions[0].allocations:
        if not isinstance(alloc, mb.MemoryLocationSet):
            continue
        name = alloc.memorylocations[0].name
        if alloc.kind == "ExternalInput":
            if name != partition_name:
                in_names.append(name)
        elif alloc.kind == "ExternalOutput":
            out_names.append(name)
            shape = tuple(alloc.tensor_shape)
            dtype = mb.dt.np(alloc.dtype)
            out_avals.append(jax.core.ShapedArray(shape, dtype))
            zero_shapes.append((shape, dtype))
    n_params = len(in_names)
    n_outs = len(out_avals)
    all_names = in_names + out_names
    if partition_name is not None:
        all_names = all_names + [partition_name]
    donate = tuple(range(n_params, n_params + n_outs))

    def _body(*args):
        operands = list(args)
        if partition_name is not None:
            operands.append(bass2jax.partition_id_tensor())
        outs = bass2jax._bass_exec_p.bind(
            *operands,
            out_avals=tuple(out_avals),
            in_names=tuple(all_names),
            out_names=tuple(out_names),
            lowering_input_output_aliases=(),
            sim_require_finite=True,
            sim_require_nnan=True,
            nc=nc,
        )
        return tuple(outs)

    devices = jax.devices()[:NCORES]
    mesh = Mesh(np.asarray(devices), ("core",))
    in_specs = (PartitionSpec("core"),) * (n_params + n_outs)
    out_specs = (PartitionSpec("core"),) * n_outs
    sharded = jax.jit(
        shard_map(
            _body, mesh=mesh, in_specs=in_specs, out_specs=out_specs,
            check_rep=False,
        ),
        donate_argnums=donate,
        keep_unused=True,
    )

    def run(in_maps):
        concat_in = [
            np.concatenate([np.asarray(m[name]) for m in in_maps], axis=0)
            for name in in_names
        ]
        concat_zeros = [
            np.zeros((NCORES * s[0], *s[1:]), dt) for s, dt in zero_shapes
        ]
        out_arrs = sharded(*concat_in, *concat_zeros)
        return [
            {
                name: np.asarray(out_arrs[i]).reshape(
                    NCORES, *out_avals[i].shape
                )[c]
                for i, name in enumerate(out_names)
            }
            for c in range(NCORES)
        ]

    _CACHE["runner"] = run
    return run


def kernel(**inputs) -> np.ndarray:
    in_maps = _shard_inputs(inputs)
    results = _get_runner()(in_maps)
    return _gather(results, inputs)


# revision 17
# speedup vs baseline: 1.7612x; 1.7612x over previous
"""Bass/Tile Trainium2 kernel for nn_ConstrainedAttention (B=2,S=2048,D=512,H=8).

Sharding: 8 cores = 2 batches x 4 head-pairs. Core c handles batch b=c//4 and
heads (2*(c%4), 2*(c%4)+1). Each core computes its heads' attention plus the
output-projection partial; the host sums the 4 partials per batch and adds bo.

Device-side layout is "scores transposed": [k on partitions, q on free dim].
Softmax is computed without a max-subtraction pass (scores are bounded, fp32
exp cannot overflow) and the softmax denominator falls out of the attn@V
matmul via an appended ones-column on V. Position bias below -POS_CUT makes
exp() vanish at fp32 precision, so score tiles entirely outside the band are
skipped (the sparse-attention structure of this problem).
"""

import sys

sys.path.insert(0, "/opt/trn_rl_repo")

import numpy as np

import bass_rust
import concourse.bass as bass
import concourse.tile as tile
from concourse import mybir
from concourse.masks import make_identity
from concourse.vector_clock import ScopedClock

# ---- problem constants (hardcoded per contract) ----
B, S, D, H, HD, DSEM = 2, 2048, 512, 8, 64, 256
P = 128
NCORES = 8
SEM_THRESH = 0.5
SEM_STRENGTH = 1.0
POS_WINDOW = 10.0
POS_DECAY = 0.1
TEMP_MIN, TEMP_MAX = 0.5, 2.0

QC = 4          # q chunks of 512
KC = S // P     # 16 k chunks of 128
QW = 512        # q chunk width
# Skip (kc,qc) score tiles whose minimum |q-k| distance puts pos_bias below
# -POS_CUT: exp(score + pos) is then < e-50 relative to the softmax sum.
POS_CUT = 40.0

F32 = mybir.dt.float32
BF16 = mybir.dt.bfloat16
F32R = mybir.dt.float32r
AX = mybir.AxisListType.X
ALU = mybir.AluOpType
ACTF = mybir.ActivationFunctionType


def _tile_kept(kc: int, qc: int) -> bool:
    """Does score tile (k in [kc*128,kc*128+128), q in [qc*512,qc*512+512))
    intersect the band where pos_bias > -POS_CUT?"""
    dmax = POS_WINDOW + POS_CUT / POS_DECAY  # distance where bias hits -POS_CUT
    k0, k1 = kc * P, kc * P + P - 1
    q0, q1 = qc * QW, qc * QW + QW - 1
    # min |q - k| over the tile
    if q0 <= k1 and k0 <= q1:
        dmin = 0
    else:
        dmin = min(abs(q0 - k1), abs(k0 - q1))
    return dmin <= dmax


def _kept_deltas() -> list[int]:
    ds = sorted({qc * QW - kc * P for qc in range(QC) for kc in range(KC)
                 if _tile_kept(kc, qc)})
    return ds


def _pos_tile(delta: int) -> np.ndarray:
    """pos_biasT tile [128 k, 512 q] for q0-k0 == delta:
    t[dk, dq] = g(delta + dq - dk), g(d) = min(0, -0.1*(|d|-10))."""
    dk = np.arange(P)[:, None]
    dq = np.arange(QW)[None, :]
    d = np.abs(delta + dq - dk).astype(np.float32)
    return np.where(d > POS_WINDOW, -POS_DECAY * (d - POS_WINDOW), 0.0).astype(
        np.float32
    )


def _bcast(ap, p):
    """Broadcast an AP along the partition dim (step 0, count p)."""
    return bass.AP(tensor=ap.tensor, offset=ap.offset, ap=[[0, p]] + ap.ap[1:])


def _patched_drain_and_barrier(self, tick_clock, wait_clock):
    """The walrus build in this container rejects >1 sem wait on TPB_CTRL
    instructions (Drain/Nop). Spread the tile-exit waits one-per-nop."""
    nop_inst = self.nc.sync.nop(nofuse=True, hint="tile_exit_wait")
    wait_clock.add_sem_waits(
        nop_inst.ins, ScopedClock({None: tick_clock.global_clock})
    )
    waits = list(nop_inst.ins.sync_info.on_wait)
    nop_inst.ins.sync_info.on_wait = waits[:1]
    for w in waits[1:]:
        extra = self.nc.sync.nop(nofuse=True, hint="tile_exit_wait")
        extra.ins.sync_info = bass_rust.SyncInfo(on_wait=[w], on_update=[])
    self.nc.sync.drain()
    self.nc.all_engine_barrier()
    popped = self.nc._tile_sem_poison_stack.pop()
    assert popped is self._sem_poison
    self.nc.clear_and_free_semaphores(list(self.sems.allocated().values()))
    self.nc.all_engine_barrier()


tile.TileContext._drain_and_barrier = _patched_drain_and_barrier


def _split_multi_waits_json(raw: bytes) -> bytes:
    """This container's walrus accepts at most ONE semaphore wait per
    instruction (setupSyncWait: 'Too many sync wait commands'). Rewrite the
    serialized BIR: for every instruction carrying N>1 waits, hoist N-1 of
    them onto same-engine NoOps inserted immediately before it."""
    import json as _json

    d = _json.loads(raw)
    seq = [0]
    for fn in d["functions"]:
        for bb in fn["blocks"]:
            new_insts = []
            for ins in bb["instructions"]:
                si = ins.get("sync_info")
                waits = (si or {}).get("on_wait") or []
                if len(waits) > 1:
                    for w in waits[:-1]:
                        seq[0] += 1
                        new_insts.append({
                            "debug": ins.get("debug", 0),
                            "engine": ins["engine"],
                            "ins": [],
                            "outs": [],
                            "name": f"I-w{seq[0]}",
                            "opcode": "NoOp",
                            "sync_info": {"on_update": [], "on_wait": [w]},
                            "text_hint": "split_wait",
                        })
                    si["on_wait"] = [waits[-1]]
                new_insts.append(ins)
            bb["instructions"] = new_insts
    return _json.dumps(d).encode()


_orig_to_json_bytes = bass.Bass.to_json_bytes


def _to_json_bytes_split(self, *a, **kw):
    return _split_multi_waits_json(_orig_to_json_bytes(self, *a, **kw))


bass.Bass.to_json_bytes = _to_json_bytes_split


def r32(x):
    return x.bitcast(F32R)


def build_nc() -> bass.Bass:
    """Build the per-core Bass program (identical on all 8 cores)."""
    nc = bass.Bass()
    deltas = _kept_deltas()
    didx = {d: i for i, d in enumerate(deltas)}
    ND = len(deltas)

    # ---- DRAM I/O ----
    qT = nc.dram_tensor("qT", [D, S], BF16, kind="ExternalInput")
    kT = nc.dram_tensor("kT", [D, S], BF16, kind="ExternalInput")
    vT = nc.dram_tensor("vT", [D, S], BF16, kind="ExternalInput")
    semT = nc.dram_tensor("semT", [DSEM, S], BF16, kind="ExternalInput")
    maskc = nc.dram_tensor("maskc", [P, KC], F32, kind="ExternalInput")
    wq = nc.dram_tensor("wq", [D, P], BF16, kind="ExternalInput")
    wk = nc.dram_tensor("wk", [D, P], BF16, kind="ExternalInput")
    wv = nc.dram_tensor("wv", [D, P], BF16, kind="ExternalInput")
    wo = nc.dram_tensor("wo", [P, D], F32, kind="ExternalInput")
    bqc = nc.dram_tensor("bqc", [P, 1], F32, kind="ExternalInput")
    bkc = nc.dram_tensor("bkc", [P, 1], F32, kind="ExternalInput")
    bvc = nc.dram_tensor("bvc", [P, 1], F32, kind="ExternalInput")
    wt1T = nc.dram_tensor("wt1T", [D, D // 2], F32, kind="ExternalInput")
    bt1r = nc.dram_tensor("bt1r", [1, D // 2], F32, kind="ExternalInput")
    wt2r = nc.dram_tensor("wt2r", [1, D // 2], F32, kind="ExternalInput")
    bt2s = nc.dram_tensor("bt2s", [1, 1], F32, kind="ExternalInput")
    posb = nc.dram_tensor("posb", [ND, P, QW], F32, kind="ExternalInput")
    out = nc.dram_tensor("out", [S, D], F32, kind="ExternalOutput")

    with nc.allow_low_precision(reason="f32r-rounded matmul inputs"), \
            tile.TileContext(nc) as tc:
        with (
            tc.tile_pool(name="persist", bufs=1) as pp,
            tc.tile_pool(name="stream", bufs=12) as st,
            tc.tile_pool(name="work", bufs=3) as wk_pool,
            tc.tile_pool(name="psum_big", bufs=2, space="PSUM") as pb,
            tc.tile_pool(name="psum_sc", bufs=2, space="PSUM") as psc,
            tc.tile_pool(name="psum_sim", bufs=2, space="PSUM") as ps,
            tc.tile_pool(name="psum_ctx", bufs=2, space="PSUM") as pc,
            tc.tile_pool(name="dram", bufs=1, space="DRAM") as dram,
        ):
            # ---- persistent SBUF ----
            sn_sb = pp.tile([P, 2, S], BF16)            # semantic.T (2 chunks)
            pos_sb = pp.tile([P, ND, QW], F32)         # pos-bias tiles by delta
            q0_sb = pp.tile([HD, S], BF16)              # head0 Q.T (scaled)
            q1_sb = pp.tile([HD, S], BF16)
            k0_sb = pp.tile([HD, S], BF16)
            k1_sb = pp.tile([HD, S], BF16)
            v0_sb = pp.tile([P, KC, HD + 1], BF16)      # V per k-chunk + ones col
            v1_sb = pp.tile([P, KC, HD + 1], BF16)
            ctx_sb = pp.tile([P, S], F32)              # both heads' ctx.T
            mask_sb = pp.tile([P, KC], F32)
            wq_sb = pp.tile([P, 4, P], BF16)
            wk_sb = pp.tile([P, 4, P], BF16)
            wv_sb = pp.tile([P, 4, P], BF16)
            wo_sb = pp.tile([P, D], F32)
            bq_sb = pp.tile([P, 1], F32)
            bk_sb = pp.tile([P, 1], F32)
            bv_sb = pp.tile([P, 1], F32)
            wt1_sb = pp.tile([P, 4, D // 2], F32)
            bt1_sb = pp.tile([1, D // 2], F32)
            wt2_sb = pp.tile([1, D // 2], F32)
            bt2_sb = pp.tile([1, 1], F32)
            ident = pp.tile([P, P], F32)
            ident_r = pp.tile([P, P], F32)
            ones_sb = pp.tile([P, 1], BF16)
            ones_row = pp.tile([1, P], F32)
            scale_col = pp.tile([P, 1], F32)
            qsum_sb = pp.tile([P, 4], F32)
            rnorm_sb = pp.tile([1, S], F32)
            scale_sb = pp.tile([1, 1], F32)            # 1/(8*temp)
            bqs_sb = pp.tile([P, 1], F32)              # bq * scale
            mlp_sb = pp.tile([1, D // 2], F32)
            tsig_sb = pp.tile([1, 1], F32)

            # ---- constant/small loads ----
            nc.sync.dma_start(out=mask_sb, in_=maskc[:])
            nc.sync.dma_start(out=wq_sb, in_=wq.rearrange("(c p) m -> p c m", p=P))
            nc.sync.dma_start(out=wk_sb, in_=wk.rearrange("(c p) m -> p c m", p=P))
            nc.sync.dma_start(out=wv_sb, in_=wv.rearrange("(c p) m -> p c m", p=P))
            nc.sync.dma_start(out=r32(wo_sb[:]), in_=r32(wo[:]))
            nc.sync.dma_start(out=bq_sb, in_=bqc[:])
            nc.sync.dma_start(out=bk_sb, in_=bkc[:])
            nc.sync.dma_start(out=bv_sb, in_=bvc[:])
            nc.sync.dma_start(
                out=r32(wt1_sb[:]), in_=r32(wt1T.rearrange("(c p) m -> p c m", p=P))
            )
            nc.sync.dma_start(out=bt1_sb, in_=bt1r[:])
            nc.sync.dma_start(out=wt2_sb, in_=wt2r[:])
            nc.sync.dma_start(out=bt2_sb, in_=bt2s[:])
            nc.sync.dma_start(
                out=sn_sb, in_=semT.rearrange("(c p) m -> p c m", p=P)
            )
            make_identity(nc, ident)
            nc.scalar.copy(r32(ident_r[:]), ident)
            nc.vector.memset(ones_sb, 1.0)
            nc.vector.memset(ones_row, 1.0)
            nc.vector.memset(v0_sb[:, :, HD : HD + 1], 1.0)
            nc.vector.memset(v1_sb[:, :, HD : HD + 1], 1.0)

            # ---- semantic feature normalization ----
            # norms^2 per column via square + ones-matmul, then rsqrt, then
            # scale sn in place.
            for qs in range(QC):
                n2 = pb.tile([1, QW], F32, tag="big", name=f"n2_{qs}")
                for c in range(2):
                    sq = wk_pool.tile([P, QW], BF16, tag="sq")
                    nc.vector.tensor_mul(
                        sq,
                        sn_sb[:, c, qs * QW : (qs + 1) * QW],
                        sn_sb[:, c, qs * QW : (qs + 1) * QW],
                    )
                    nc.tensor.matmul(
                        n2, ones_sb, sq, start=(c == 0), stop=(c == 1)
                    )
                nrm = wk_pool.tile([1, QW], F32, tag="nrm")
                nc.scalar.activation(nrm, n2, ACTF.Sqrt)
                nc.vector.reciprocal(r32(rnorm_sb[:, qs * QW : (qs + 1) * QW]), nrm)
            for qs in range(QC):
                qsl = slice(qs * QW, (qs + 1) * QW)
                rb = pb.tile([P, QW], F32, tag="big", name=f"rb{qs}")
                nc.tensor.matmul(
                    rb, r32(ones_row), r32(rnorm_sb[:, qsl]),
                    start=True, stop=True,
                )
                for c in range(2):
                    nc.vector.tensor_mul(sn_sb[:, c, qsl], sn_sb[:, c, qsl], rb)

            # ---- Q/K/V projections (stream x.T chunks over D) ----
            # Q also feeds the temperature MLP via per-chunk row sums.
            xqs = [st.tile([P, S], BF16, tag="xT", name=f"xq{i}") for i in range(4)]
            for kc in range(4):
                nc.sync.dma_start(out=xqs[kc], in_=qT[kc * P : (kc + 1) * P, :])
                nc.vector.reduce_sum(r32(qsum_sb[:, kc : kc + 1]), xqs[kc], axis=AX)

            # temperature MLP: sigmoid(relu(qm@Wt1.T+bt1)@Wt2.T+bt2)
            h1p = pb.tile([1, D // 2], F32, tag="big")
            for kc in range(4):
                nc.tensor.matmul(
                    h1p,
                    r32(qsum_sb[:, kc : kc + 1]),
                    r32(wt1_sb[:, kc, :]),
                    start=(kc == 0),
                    stop=(kc == 3),
                )
            nc.vector.tensor_scalar(
                mlp_sb, h1p, 1.0 / S, None, op0=ALU.mult
            )
            nc.vector.tensor_add(mlp_sb, mlp_sb, bt1_sb)
            nc.scalar.activation(mlp_sb, mlp_sb, ACTF.Relu)
            nc.vector.tensor_mul(mlp_sb, mlp_sb, wt2_sb)
            nc.vector.reduce_sum(tsig_sb, mlp_sb, axis=AX)
            nc.scalar.activation(tsig_sb, tsig_sb, ACTF.Sigmoid, bias=bt2_sb)
            # scale = 1/(sqrt(HD)*temp) = 1/(8*(0.5+1.5*sig)) = 1/(12*sig+4)
            nc.vector.tensor_scalar(
                tsig_sb, tsig_sb, 12.0, 4.0, op0=ALU.mult, op1=ALU.add
            )
            nc.vector.reciprocal(r32(scale_sb[:]), tsig_sb)
            # broadcast the scalar to all partitions via a DRAM bounce (SBUF
            # sources cannot have partition-step-0 APs; DRAM sources can)
            scale_dr = dram.tile([1, 1], F32)
            nc.sync.dma_start(out=scale_dr, in_=scale_sb)
            nc.sync.dma_start(out=scale_col, in_=_bcast(scale_dr[:], P))
            nc.vector.tensor_scalar(
                bqs_sb, bq_sb, scale_col, None, op0=ALU.mult
            )

            # Q = x@Wq per q-chunk; evict with (x + bq) * scale fused:
            # out = in*scale + bq*scale
            for qs in range(QC):
                qp = pb.tile([P, QW], F32, tag="big", name=f"qp{qs}")
                for kc in range(4):
                    nc.tensor.matmul(
                        qp,
                        wq_sb[:, kc, :],
                        xqs[kc][:, qs * QW : (qs + 1) * QW],
                        start=(kc == 0),
                        stop=(kc == 3),
                    )
                nc.scalar.activation(
                    q0_sb[:, qs * QW : (qs + 1) * QW],
                    qp[0:HD, :],
                    ACTF.Identity,
                    bias=bqs_sb[0:HD, :],
                    scale=scale_col[0:HD, :],
                )
                nc.scalar.activation(
                    q1_sb[:, qs * QW : (qs + 1) * QW],
                    qp[HD:P, :],
                    ACTF.Identity,
                    bias=bqs_sb[HD:P, :],
                    scale=scale_col[HD:P, :],
                )

            xks = [st.tile([P, S], BF16, tag="xT", name=f"xk{i}") for i in range(4)]
            for kc in range(4):
                nc.sync.dma_start(out=xks[kc], in_=kT[kc * P : (kc + 1) * P, :])
            for qs in range(QC):
                kp = pb.tile([P, QW], F32, tag="big", name=f"kp{qs}")
                for kc in range(4):
                    nc.tensor.matmul(
                        kp,
                        wk_sb[:, kc, :],
                        xks[kc][:, qs * QW : (qs + 1) * QW],
                        start=(kc == 0),
                        stop=(kc == 3),
                    )
                nc.scalar.activation(
                    k0_sb[:, qs * QW : (qs + 1) * QW],
                    kp[0:HD, :],
                    ACTF.Identity,
                    bias=bk_sb[0:HD, :],
                )
                nc.scalar.activation(
                    k1_sb[:, qs * QW : (qs + 1) * QW],
                    kp[HD:P, :],
                    ACTF.Identity,
                    bias=bk_sb[HD:P, :],
                )

            # V: project to vT layout then transpose per 128-chunk into
            # [k-part, head-dim] with the ones column for the softmax sum.
            vtmp_sb = pp.tile([P, S], F32)
            xvs = [st.tile([P, S], BF16, tag="xT", name=f"xv{i}") for i in range(4)]
            for kc in range(4):
                nc.sync.dma_start(out=xvs[kc], in_=vT[kc * P : (kc + 1) * P, :])
            for qs in range(QC):
                vp = pb.tile([P, QW], F32, tag="big", name=f"vp{qs}")
                for kc in range(4):
                    nc.tensor.matmul(
                        vp,
                        wv_sb[:, kc, :],
                        xvs[kc][:, qs * QW : (qs + 1) * QW],
                        start=(kc == 0),
                        stop=(kc == 3),
                    )
                nc.scalar.activation(
                    vtmp_sb[:, qs * QW : (qs + 1) * QW],
                    vp,
                    ACTF.Identity,
                    bias=bv_sb,
                )
            for sc in range(KC):
                vtp = pb.tile([P, P], F32, tag="big", name=f"vtp{sc}")
                nc.tensor.transpose(
                    vtp, vtmp_sb[:, sc * P : (sc + 1) * P], ident
                )
                nc.scalar.copy(v0_sb[:, sc, 0:HD], vtp[:, 0:HD])
                nc.scalar.copy(v1_sb[:, sc, 0:HD], vtp[:, HD:P])

            # ---- main attention loop ----
            nc.sync.dma_start(
                out=r32(pos_sb[:]), in_=r32(posb.rearrange("n p m -> p n m"))
            )
            for qc in range(QC):
                cx0 = pc.tile([HD + 1, QW], F32, tag="ctx")
                cx1 = pc.tile([HD + 1, QW], F32, tag="ctx")
                kept = [kc for kc in range(KC) if _tile_kept(kc, qc)]
                for kc in kept:
                    first = kc == kept[0]
                    last = kc == kept[-1]
                    d = qc * QW - kc * P
                    qsl = slice(qc * QW, (qc + 1) * QW)
                    ksl = slice(kc * P, (kc + 1) * P)
                    # semantic bias tile: min(sim-0.5, 0) + pos
                    smp = ps.tile([P, QW], F32, tag="sim")
                    for c in range(2):
                        nc.tensor.matmul(
                            smp,
                            sn_sb[:, c, ksl],
                            sn_sb[:, c, qsl],
                            start=(c == 0),
                            stop=(c == 1),
                        )
                    bias = wk_pool.tile([P, QW], F32, tag="bias")
                    nc.vector.tensor_scalar(
                        r32(bias[:]), smp, SEM_THRESH, SEM_THRESH,
                        op0=ALU.min, op1=ALU.subtract,
                    )
                    nc.vector.tensor_add(r32(bias[:]), bias, pos_sb[:, didx[d], :])
                    for h, (qh, kh, vh, cx) in enumerate(
                        ((q0_sb, k0_sb, v0_sb, cx0), (q1_sb, k1_sb, v1_sb, cx1))
                    ):
                        scp = psc.tile([P, QW], F32, tag="sc")
                        nc.tensor.matmul(
                            scp, kh[:, ksl], qh[:, qsl],
                            start=True, stop=False,
                        )
                        nc.tensor.matmul(
                            scp, r32(ident_r), r32(bias),
                            start=False, stop=True,
                        )
                        ee = wk_pool.tile([P, QW], BF16, tag="ee")
                        nc.scalar.activation(
                            ee, scp, ACTF.Exp, bias=mask_sb[:, kc : kc + 1]
                        )
                        nc.tensor.matmul(
                            cx, vh[:, kc, :], ee,
                            start=first, stop=last,
                        )
                # normalize: ctx /= sum (sum = ones-row of the V matmul)
                for h, cx in enumerate((cx0, cx1)):
                    ub = wk_pool.tile([HD + 1, QW], F32, tag="ub")
                    nc.scalar.copy(ub, cx)  # frees the PSUM accumulator fast
                    rec = wk_pool.tile([1, QW], F32, tag="rec")
                    nc.vector.reciprocal(r32(rec[:]), ub[HD : HD + 1, :])
                    rcb = ps.tile([HD, QW], F32, tag="sim", name=f"rcb{qc}_{h}")
                    nc.tensor.matmul(
                        rcb, r32(ones_row[:, 0:HD]), r32(rec),
                        start=True, stop=True,
                    )
                    nc.vector.tensor_mul(
                        r32(ctx_sb[h * HD : (h + 1) * HD, qc * QW : (qc + 1) * QW]),
                        ub[0:HD, :],
                        rcb,
                    )
                # output-projection partial for this q-chunk's s rows
                for sc in range(4 * qc, 4 * qc + 4):
                    op = pb.tile([P, D], F32, tag="big", name=f"op{sc}")
                    nc.tensor.matmul(
                        op,
                        r32(ctx_sb[:, sc * P : (sc + 1) * P]),
                        r32(wo_sb),
                        start=True,
                        stop=True,
                    )
                    ob = wk_pool.tile([P, D], F32, tag="ob")
                    nc.vector.tensor_copy(ob, op)
                    nc.sync.dma_start(out=out[sc * P : (sc + 1) * P, :], in_=ob)

    return nc


# ---------------------------------------------------------------- host side

_CACHE: dict = {}


def _shard_inputs(inputs: dict) -> list[dict]:
    f = np.float32
    q = np.asarray(inputs["query"], f)
    k = np.asarray(inputs["key"], f)
    v = np.asarray(inputs["value"], f)
    mask = np.asarray(inputs["mask"])
    sem = np.asarray(inputs["semantic_features"], f)
    Wq, bq = np.asarray(inputs["Wq"], f), np.asarray(inputs["bq"], f)
    Wk, bk = np.asarray(inputs["Wk"], f), np.asarray(inputs["bk"], f)
    Wv, bv = np.asarray(inputs["Wv"], f), np.asarray(inputs["bv"], f)
    Wo = np.asarray(inputs["Wo"], f)
    Wt1, bt1 = np.asarray(inputs["Wt1"], f), np.asarray(inputs["bt1"], f)
    Wt2, bt2 = np.asarray(inputs["Wt2"], f), np.asarray(inputs["bt2"], f)

    deltas = _kept_deltas()
    posb = np.stack([_pos_tile(d) for d in deltas])  # [ND,128,512]
    wt1T = np.ascontiguousarray(Wt1.T)               # [512,256]

    import ml_dtypes

    bf16 = ml_dtypes.bfloat16
    maps = []
    for c in range(NCORES):
        b, hp = divmod(c, 4)
        cols = slice(P * hp, P * hp + P)
        maskbias = np.where(mask[b] == 0, np.float32(-1e30), np.float32(0.0))
        maps.append({
            "qT": np.ascontiguousarray(q[b].T).astype(bf16),
            "kT": np.ascontiguousarray(k[b].T).astype(bf16),
            "vT": np.ascontiguousarray(v[b].T).astype(bf16),
            "semT": np.ascontiguousarray(sem[b].T).astype(bf16),
            "maskc": np.ascontiguousarray(maskbias.reshape(KC, P).T),
            "wq": np.ascontiguousarray(Wq[cols, :].T).astype(bf16),
            "wk": np.ascontiguousarray(Wk[cols, :].T).astype(bf16),
            "wv": np.ascontiguousarray(Wv[cols, :].T).astype(bf16),
            "wo": np.ascontiguousarray(Wo[:, cols].T),
            "bqc": np.ascontiguousarray(bq[cols].reshape(P, 1)),
            "bkc": np.ascontiguousarray(bk[cols].reshape(P, 1)),
            "bvc": np.ascontiguousarray(bv[cols].reshape(P, 1)),
            "wt1T": wt1T,
            "bt1r": bt1.reshape(1, -1),
            "wt2r": Wt2.reshape(1, -1),
            "bt2s": bt2.reshape(1, 1),
            "posb": posb,
        })
    return maps


def _gather(results: list[dict], inputs: dict) -> np.ndarray:
    bo = np.asarray(inputs["bo"], np.float32)
    out = np.zeros((B, S, D), np.float32)
    for c in range(NCORES):
        out[c // 4] += results[c]["out"]
    out += bo
    return out


def get_nc() -> bass.Bass:
    if "nc" not in _CACHE:
        _CACHE["nc"] = build_nc()
    return _CACHE["nc"]


def _get_runner():
    """Compile once and return a reusable 8-core runner (the stock
    run_bass_via_pjrt re-jits per call, recompiling the NEFF each time)."""
    if "runner" in _CACHE:
        return _CACHE["runner"]

    import jax
    from jax.sharding import Mesh, PartitionSpec
    from jax.experimental.shard_map import shard_map
    from concourse import bass2jax, mybir as mb

    nc = get_nc()
    bass2jax.install_neuronx_cc_hook()

    in_names, out_names, out_avals, zero_shapes = [], [], [], []
    partition_name = (
        nc.partition_id_tensor.name if nc.partition_id_tensor else None
    )
    for alloc in nc.m.functions[0].allocations:
        if not isinstance(alloc, mb.MemoryLocationSet):
            continue
        name = alloc.memorylocations[0].name
        if alloc.kind == "ExternalInput":
            if name != partition_name:
                in_names.append(name)
        elif alloc.kind == "ExternalOutput":
            out_names.append(name)
            shape = tuple(alloc.tensor_shape)
            dtype = mb.dt.np(alloc.dtype)
            out_avals.append(jax.core.ShapedArray(shape, dtype))
            zero_shapes.append((shape, dtype))
    n_params = len(in_names)
    n_outs = len(out_avals)
    all_names = in_names + out_names
    if partition_name is not None:
        all_names = all_names + [partition_name]
    donate = tuple(range(n_params, n_params + n_outs))

    def _body(*args):
        operands = list(args)
        if partition_name is not None:
            operands.append(bass2jax.partition_id_tensor())
        outs = bass2jax._bass_exec_p.bind(
            *operands,
            out_avals=tuple(out_avals),
            in_names=tuple(all_names),
            out_names=tuple(out_names),
            lowering_input_output_aliases=(),
            sim_require_finite=True,
            sim_require_nnan=True,
            nc=nc,
        )
        return tuple(outs)

    devices = jax.devices()[:NCORES]
    mesh = Mesh(np.asarray(devices), ("core",))
    in_specs = (PartitionSpec("core"),) * (n_params + n_outs)
    out_specs = (PartitionSpec("core"),) * n_outs
    sharded = jax.jit(
        shard_map(
            _body, mesh=mesh, in_specs=in_specs, out_specs=out_specs,
            check_rep=False,
        ),
        donate_argnums=donate,
        keep_unused=True,
    )

    def run(in_maps):
        concat_in = [
            np.concatenate([np.asarray(m[name]) for m in in_maps], axis=0)
            for name in in_names
        ]
        concat_zeros = [
            np.zeros((NCORES * s[0], *s[1:]), dt) for s, dt in zero_shapes
        ]
        out_arrs = sharded(*concat_in, *concat_zeros)
        return [
            {
                name: np.asarray(out_arrs[i]).reshape(
                    NCORES, *out_avals[i].shape
                )[c]
                for i, name in enumerate(out_names)
            }
            for c in range(NCORES)
        ]

    _CACHE["runner"] = run
    return run


def kernel(**inputs) -> np.ndarray:
    in_maps = _shard_inputs(inputs)
    results = _get_runner()(in_maps)
    return _gather(results, inputs)
